# revision 20
# baseline (speedup 1.0000x reference)
"""Trainium2 Bass kernel for CustomSNNLoss (nn_CustomSNNLoss_36429912604816).

Strategy v2 (sorted-class block-diagonal decomposition):
  - Host: sort rows by (target, batch) so classes/combos are contiguous key
    ranges. Normalize x, pre-transpose to xnT [D=128, N].
  - Each of 8 cores owns R = 768 query rows (contiguous in sorted order) and
    all 6144 keys. Key tiles are ROTATED per core so the core's block-diagonal
    key range (keys sharing a target class with the core's queries) always
    sits at tiles [0, DSPAN) -- one uniform SPMD program, per-core data.
  - Per (chunk of 384 queries) x (group of 3 key tiles):
      sim  = xnT[ktile].T @ xnq          (f32r matmul -> PSUM, 3 banks/group)
      one Act instr: St = exp(sim/t) (bf16) spanning all 3 banks
        (diag groups: Sb = exp(2*sim), DVE squares -> St when t == 0.25)
      PE: rowsum via ones-weights matmul on every tile; class-slot (St) and
          combo-slot (Sb) one-hot matmuls on the DSPAN diag tiles only.
    Accumulators live in one PSUM bank: rows [0:1+NCLS] = ones+class slots
    (St), rows [1+NCLS:] = combo slots (Sb).
  - Host epilogue (O(N)): decode slots, per-row pos/neg sums, -log losses,
    validity masks, class-weighted means, final scalar.
"""

import numpy as np

N, D = 6144, 128
P = 128                 # partitions / contraction tile
NCORES = 8
R = N // NCORES         # 768 query rows per core
KT = N // P             # 48 key tiles
QC = 2                  # query chunks per core
QF = R // QC            # 384 free-dim per matmul
GRP = 3                 # key tiles per PSUM/Act group
NG = KT // GRP          # 16 groups per chunk
NT, NB = 20, 5          # target classes, batch keys
MIN_T, MAX_T = 0.1, 1.0
TEMP_BATCH = 0.5
EPS = 1e-8

_compile_cache = {}
LAST_RESULT = None  # BassKernelResults from the most recent device run


def _build(scale_t: float, scale_b: float, square_mode: bool,
           dspan: int, ncls: int):
    """dspan: #rotated key tiles with class/combo matmuls (multiple of GRP).
    ncls: number of class slots per chunk. Acc rows = 1 + ncls + 5*ncls."""
    from contextlib import ExitStack

    import concourse.bacc as bacc
    import concourse.mybir as mybir
    import concourse.tile as tile

    f32 = mybir.dt.float32
    bf16 = mybir.dt.bfloat16
    EXP = mybir.ActivationFunctionType.Exp

    nwt = 1 + ncls            # ones col + class slots
    nwb = 5 * ncls            # combo slots
    nacc = 32 + nwb           # combo region lives at partition base 32
    dgrp = dspan // GRP       # diag groups per chunk

    nc = bacc.Bacc("TRN2", target_bir_lowering=False, debug=False,
                   enable_asserts=False)

    xnt = nc.dram_tensor("xnt", [P, N], bf16, kind="ExternalInput").ap()
    xnq = nc.dram_tensor("xnq", [P, R], bf16, kind="ExternalInput").ap()
    # per (chunk, diag tile): [P, nwt] ones+class | [P, nwb] combo weights
    wt = nc.dram_tensor("wt", [P, QC * dspan * nwt], bf16,
                        kind="ExternalInput").ap()
    wb = nc.dram_tensor("wb", [P, QC * dspan * nwb], bf16,
                        kind="ExternalInput").ap()
    out = nc.dram_tensor("out", [nacc, R], f32, kind="ExternalOutput").ap()

    with tile.TileContext(nc) as tc, ExitStack() as ctx:
        const = ctx.enter_context(tc.tile_pool(name="const", bufs=1))
        work = ctx.enter_context(tc.tile_pool(name="work", bufs=3))
        psim = ctx.enter_context(tc.tile_pool(name="psim", bufs=2,
                                              space="PSUM"))
        pacc = ctx.enter_context(tc.tile_pool(name="pacc", bufs=2,
                                              space="PSUM"))

        xnT_sb = const.tile([P, N], bf16, name="xnT_sb")
        xnq_sb = const.tile([P, R], bf16, name="xnq_sb")
        wt_sb = const.tile([P, QC * dspan * nwt], bf16, name="wt_sb")
        wb_sb = const.tile([P, QC * dspan * nwb], bf16, name="wb_sb")
        # col 0 = ones (rowsum), cols 1.. = zeros (keep class rows untouched
        # while matmuls on off-diag tiles still target partition base 0)
        ones_sb = const.tile([P, nwt], bf16, name="ones_sb")
        out_sb = const.tile([nacc, R], f32, name="out_sb")

        nc.vector.memset(ones_sb[:], 0.0)
        nc.vector.memset(ones_sb[:, 0:1], 1.0)

        # Loads: k-loop consumes xnt tiles in rotated order 0..47; queries,
        # weights needed from k=0.
        # first keys ASAP (k-loop consumes rotated tiles in order), weights
        # next (first acc matmul needs them ~2us in), bulk keys last
        nc.sync.dma_start(xnq_sb[:], xnq[:])
        nc.sync.dma_start(xnT_sb[:, 0:768], xnt[:, 0:768])
        nc.sync.dma_start(wt_sb[:], wt[:])
        nc.sync.dma_start(wb_sb[:], wb[:])
        for lo, hi in ((768, 2048), (2048, 3584), (3584, 6144)):
            nc.sync.dma_start(xnT_sb[:, lo:hi], xnt[:, lo:hi])

        def acc_mms(q, g, acc, st3, sb3):
            for j in range(GRP):
                k = g * GRP + j
                if k < dspan:
                    woff = (q * dspan + k) * nwt
                    boff = (q * dspan + k) * nwb
                    nc.tensor.matmul(acc[0:nwt, :],
                                     wt_sb[:, woff:woff + nwt],
                                     st3[:, j, :],
                                     start=(k == 0), stop=False)
                    nc.tensor.matmul(acc[32:nacc, :],
                                     wb_sb[:, boff:boff + nwb],
                                     sb3[:, j, :],
                                     start=(k == 0),
                                     stop=(k == dspan - 1))
                else:
                    nc.tensor.matmul(acc[0:nwt, :], ones_sb[:],
                                     st3[:, j, :],
                                     start=False, stop=(k == KT - 1))

        # Software pipeline: group g's accumulate matmuls are issued after
        # group g+1's sim matmuls + activation, so the PE streams
        # continuously (stays ramped) while Act runs in parallel.
        pending = None
        accs = []
        for q in range(QC):
            qsl = slice(q * QF, (q + 1) * QF)
            acc = pacc.tile([nacc, QF], f32, tag="acc", name="acc")
            accs.append(acc)
            for g in range(NG):
                sim3 = psim.tile([P, GRP, 512], f32, tag="sim", name="sim3")
                for j in range(GRP):
                    k = g * GRP + j
                    ksl = slice(k * P, (k + 1) * P)
                    nc.tensor.matmul(sim3[:, j, 0:QF], xnT_sb[:, ksl],
                                     xnq_sb[:, qsl], start=True, stop=True)
                diag = g < dgrp
                st3 = work.tile([P, GRP, QF], bf16, tag="st3", name="st3")
                sb3 = None
                if diag:
                    sb3 = work.tile([P, GRP, QF], bf16, tag="sb3", name="sb3")
                    nc.scalar.activation(sb3[:, :, :], sim3[:, :, 0:QF],
                                         EXP, scale=scale_b)
                    if square_mode:
                        nc.vector.tensor_mul(st3[:, :, :], sb3[:, :, :],
                                             sb3[:, :, :])
                    else:
                        nc.scalar.activation(st3[:, :, :], sim3[:, :, 0:QF],
                                             EXP, scale=scale_t)
                else:
                    nc.scalar.activation(st3[:, :, :], sim3[:, :, 0:QF],
                                         EXP, scale=scale_t)
                if pending is not None:
                    acc_mms(*pending)
                pending = (q, g, acc, st3, sb3)
        acc_mms(*pending)
        for q in range(QC):
            qsl = slice(q * QF, (q + 1) * QF)
            nc.vector.tensor_copy(out_sb[:, qsl], accs[q][:])
            nc.sync.dma_start(out[:, qsl], out_sb[:, qsl])

    nc.compile()
    return nc


def _get_compiled(scale_t, scale_b, square_mode, dspan, ncls):
    key = (round(scale_t, 9), round(scale_b, 9), square_mode, dspan, ncls)
    if key not in _compile_cache:
        _compile_cache[key] = _build(scale_t, scale_b, square_mode,
                                     dspan, ncls)
    return _compile_cache[key]


def _round_f32r(v):
    """Round fp32 mantissa to 11 explicit bits (the PE's FP32r format)."""
    b = np.ascontiguousarray(v, dtype=np.float32).view(np.uint32).astype(np.uint64)
    r = ((b + np.uint64(1 << 11)) >> np.uint64(12)) << np.uint64(12)
    return r.astype(np.uint32).view(np.float32)


def _host_prep(input, temperature, targets, batch0):
    import ml_dtypes

    x = np.asarray(input, dtype=np.float32)
    t = float(np.clip(np.float32(temperature), MIN_T, MAX_T))
    scale_t = 1.0 / t
    scale_b = 1.0 / TEMP_BATCH
    square_mode = abs(scale_t - 2.0 * scale_b) < 1e-6

    tg0 = np.asarray(targets).astype(np.int64)
    bt0 = np.asarray(batch0).astype(np.int64)
    perm = np.lexsort((bt0, tg0))
    tg = tg0[perm]
    bt = bt0[perm]
    x = x[perm]

    norms = np.sqrt((x * x).sum(axis=1, keepdims=True, dtype=np.float32))
    norms = np.maximum(norms, np.float32(EPS)).astype(np.float32)
    xn = (x / norms).astype(ml_dtypes.bfloat16)         # device matmul operand
    s_ii = (xn.astype(np.float32) ** 2).sum(axis=1, dtype=np.float32)

    # class start/end in sorted order
    cls_start = np.searchsorted(tg, np.arange(NT), side="left")
    cls_end = np.searchsorted(tg, np.arange(NT), side="right")

    # per-core rotation + diag span
    rots, spans = [], []
    for c in range(NCORES):
        lo_cls = tg[c * R]
        hi_cls = tg[(c + 1) * R - 1]
        d0 = int(cls_start[lo_cls]) // P
        d1 = -(-int(cls_end[hi_cls]) // P)
        rots.append(d0)
        spans.append(d1 - d0)
    dspan = -(-max(spans) // GRP) * GRP
    assert dspan <= KT // 2, f"diag span {dspan} too large"

    # per-chunk class slots
    ncls = 1
    chunk_cls = []          # list over (core, chunk) of class id list
    for c in range(NCORES):
        for q in range(QC):
            seg = tg[c * R + q * QF: c * R + (q + 1) * QF]
            cl = sorted(set(seg.tolist()))
            chunk_cls.append(cl)
            ncls = max(ncls, len(cl))

    # per-core device inputs (bf16): xnT in SBUF layout [128(d), N(keys)]
    xnq_all = np.ascontiguousarray(xn.T)                # [128, N]

    nwt = 1 + ncls
    nwb = 5 * ncls
    in_maps = []
    for c in range(NCORES):
        rot = rots[c]
        # key columns in rotated tile order, already transposed [d, keys]
        colord = ((rot * P + np.arange(N)) % N)
        xnt_c = np.ascontiguousarray(xnq_all[:, colord])
        wt_c = np.zeros((P, QC * dspan * nwt), dtype=np.float32)
        wb_c = np.zeros((P, QC * dspan * nwb), dtype=np.float32)
        for q in range(QC):
            cl = chunk_cls[c * QC + q]
            for k in range(dspan):
                gt = (rot + k) % KT
                ktg = tg[gt * P:(gt + 1) * P]
                kbt = bt[gt * P:(gt + 1) * P]
                woff = (q * dspan + k) * nwt
                boff = (q * dspan + k) * nwb
                wt_c[:, woff] = 1.0
                for si, cid in enumerate(cl):
                    m = ktg == cid
                    wt_c[m, woff + 1 + si] = 1.0
                    for b in range(NB):
                        wb_c[m & (kbt == b), boff + si * 5 + b] = 1.0
        in_maps.append({
            "xnt": xnt_c,
            "xnq": np.ascontiguousarray(xnq_all[:, c * R:(c + 1) * R]),
            "wt": wt_c.astype(ml_dtypes.bfloat16),
            "wb": wb_c.astype(ml_dtypes.bfloat16),
        })

    meta = dict(tg=tg, bt=bt, s_ii=s_ii, chunk_cls=chunk_cls, ncls=ncls,
                dspan=dspan, scale_t=scale_t, scale_b=scale_b,
                square_mode=square_mode)
    return in_maps, meta


def _epilogue(acc, meta, weight_target, weight_batch0):
    """acc: [NCORES, nacc, R] device slot sums. Everything here is O(N)."""
    f = np.float64
    tg, bt, s_ii = meta["tg"], meta["bt"], meta["s_ii"]
    ncls = meta["ncls"]
    nacc = 32 + 5 * ncls      # combo region at partition base 32
    scale_t, scale_b = meta["scale_t"], meta["scale_b"]

    nchunk = NCORES * QC
    ch = acc.reshape(NCORES, nacc, QC, QF).transpose(
        0, 2, 1, 3).reshape(nchunk, nacc, QF)

    # slot row per sorted query
    ci = np.repeat(np.arange(nchunk), QF)           # chunk index
    col = np.tile(np.arange(QF), nchunk)
    si = np.zeros(N, dtype=np.int64)
    for ic in range(nchunk):
        cl = meta["chunk_cls"][ic]
        lut = np.full(NT, -1, dtype=np.int64)
        for s, cid in enumerate(cl):
            lut[cid] = s
        seg = slice(ic * QF, (ic + 1) * QF)
        si[seg] = lut[tg[seg]]
    assert (si >= 0).all()

    rowsum_t = ch[ci, 0, col].astype(f)
    possum_t = ch[ci, 1 + si, col].astype(f)                    # incl self
    own_tb = ch[ci, 32 + si * 5 + bt, col].astype(f)            # incl self
    samet_b = ch[ci][np.arange(N)[:, None],
                     (32 + si * 5)[:, None] + np.arange(5)[None, :],
                     col[:, None]].sum(axis=1, dtype=f)

    diag_t = np.exp(scale_t * s_ii.astype(f))
    diag_b = np.exp(scale_b * s_ii.astype(f))

    cnt_t = np.bincount(tg, minlength=NT)
    n_tb = np.zeros((NT, NB), dtype=np.int64)
    np.add.at(n_tb, (tg, bt), 1)

    # ---- target SNN loss ----
    pos_t = possum_t - diag_t
    neg_t = rowsum_t - possum_t
    cnt_pos = cnt_t[tg]
    cnt_neg = N - cnt_pos
    valid_t = (cnt_pos >= 2) & (cnt_neg >= 1)
    pos_s = np.where(valid_t, pos_t, 1.0)
    neg_s = np.where(valid_t, neg_t, 1.0)
    loss_i = -np.log(pos_s / (pos_s + neg_s))
    lsum = np.bincount(tg, weights=np.where(valid_t, loss_i, 0.0), minlength=NT)
    vcnt = np.bincount(tg, weights=valid_t.astype(f), minlength=NT)
    mean = lsum / np.maximum(vcnt, 1.0)
    wt_w = np.asarray(weight_target).astype(f)
    loss_target = np.where(vcnt > 0, mean * wt_w, 0.0).sum()

    # ---- batch-effect loss ----
    pos_b = own_tb - diag_b
    neg_b = samet_b - own_tb
    cnt_pos_b = n_tb[tg, bt]
    cnt_neg_b = cnt_t[tg] - cnt_pos_b
    valid_b = (cnt_pos_b >= 2) & (cnt_neg_b >= 1)
    pos_bs = np.where(valid_b, pos_b, 1.0)
    neg_bs = np.where(valid_b, neg_b, 1.0)
    loss_bi = -np.log(pos_bs / (pos_bs + neg_bs))
    inv = np.where(valid_b, 1.0 / np.where(valid_b, loss_bi, 1.0), 0.0)
    lsum_b = np.bincount(bt, weights=inv, minlength=NB)
    vcnt_b = np.bincount(bt, weights=valid_b.astype(f), minlength=NB)
    mean_b = lsum_b / np.maximum(vcnt_b, 1.0)
    wb_w = np.asarray(weight_batch0).astype(f)
    loss_batch = np.where(vcnt_b > 0, mean_b * wb_w, 0.0).sum()

    return np.float32(0.9 * loss_target + 0.1 * loss_batch)


def _run_with_retry(nc, in_maps, core_ids, attempts=3):
    import time as _time

    from concourse.bass_utils import run_bass_kernel_spmd

    for i in range(attempts):
        try:
            return run_bass_kernel_spmd(nc, in_maps, core_ids)
        except Exception:
            if i == attempts - 1:
                raise
            _time.sleep(90)  # transient NRT device errors clear after a pause


def kernel(input, temperature, weight_target, weight_batch0, targets, batch0):
    global LAST_RESULT

    in_maps, meta = _host_prep(input, temperature, targets, batch0)
    nc = _get_compiled(meta["scale_t"], meta["scale_b"], meta["square_mode"],
                       meta["dspan"], meta["ncls"])

    LAST_RESULT = _run_with_retry(nc, in_maps, list(range(NCORES)))
    acc = np.stack([LAST_RESULT.results[c]["out"] for c in range(NCORES)])

    return _epilogue(acc, meta, weight_target, weight_batch0)


# revision 23
# speedup vs baseline: 1.1738x; 1.1738x over previous
"""Trainium2 Bass kernel for CustomSNNLoss (nn_CustomSNNLoss_36429912604816).

Strategy v2 (sorted-class block-diagonal decomposition):
  - Host: sort rows by (target, batch) so classes/combos are contiguous key
    ranges. Normalize x, pre-transpose to xnT [D=128, N].
  - Each of 8 cores owns R = 768 query rows (contiguous in sorted order) and
    all 6144 keys. Key tiles are ROTATED per core so the core's block-diagonal
    key range (keys sharing a target class with the core's queries) always
    sits at tiles [0, DSPAN) -- one uniform SPMD program, per-core data.
  - Per (chunk of 384 queries) x (group of 3 key tiles):
      sim  = xnT[ktile].T @ xnq          (f32r matmul -> PSUM, 3 banks/group)
      one Act instr: St = exp(sim/t) (bf16) spanning all 3 banks
        (diag groups: Sb = exp(2*sim), DVE squares -> St when t == 0.25)
      PE: rowsum via ones-weights matmul on every tile; class-slot (St) and
          combo-slot (Sb) one-hot matmuls on the DSPAN diag tiles only.
    Accumulators live in one PSUM bank: rows [0:1+NCLS] = ones+class slots
    (St), rows [1+NCLS:] = combo slots (Sb).
  - Host epilogue (O(N)): decode slots, per-row pos/neg sums, -log losses,
    validity masks, class-weighted means, final scalar.
"""

import numpy as np

N, D = 6144, 128
P = 128                 # partitions / contraction tile
NCORES = 8
R = N // NCORES         # 768 query rows per core
KT = N // P             # 48 key tiles
QC = 2                  # query chunks per core
QF = R // QC            # 384 free-dim per matmul
GRP = 3                 # key tiles per PSUM/Act group
NG = KT // GRP          # 16 groups per chunk
NT, NB = 20, 5          # target classes, batch keys
MIN_T, MAX_T = 0.1, 1.0
TEMP_BATCH = 0.5
EPS = 1e-8

_compile_cache = {}
LAST_RESULT = None  # BassKernelResults from the most recent device run


def _build(scale_t: float, scale_b: float, square_mode: bool,
           dspan: int, ncls: int):
    """dspan: #rotated key tiles with class/combo matmuls (multiple of GRP).
    ncls: number of class slots per chunk. Acc rows = 1 + ncls + 5*ncls."""
    from contextlib import ExitStack

    import concourse.bacc as bacc
    import concourse.mybir as mybir
    import concourse.tile as tile

    f32 = mybir.dt.float32
    bf16 = mybir.dt.bfloat16
    EXP = mybir.ActivationFunctionType.Exp

    nwt = 1 + ncls            # ones col + class slots
    nwb = 5 * ncls            # combo slots
    nacc = 32 + nwb           # combo region lives at partition base 32
    dgrp = dspan // GRP       # diag groups per chunk

    nc = bacc.Bacc("TRN2", target_bir_lowering=False, debug=False,
                   enable_asserts=False)

    xnt = nc.dram_tensor("xnt", [P, N], bf16, kind="ExternalInput").ap()
    xnq = nc.dram_tensor("xnq", [P, R], bf16, kind="ExternalInput").ap()
    # per (chunk, diag tile): [P, nwt] ones+class | [P, nwb] combo weights
    wt = nc.dram_tensor("wt", [P, QC * dspan * nwt], bf16,
                        kind="ExternalInput").ap()
    wb = nc.dram_tensor("wb", [P, QC * dspan * nwb], bf16,
                        kind="ExternalInput").ap()
    out = nc.dram_tensor("out", [nacc, R], f32, kind="ExternalOutput").ap()

    with tile.TileContext(nc) as tc, ExitStack() as ctx:
        const = ctx.enter_context(tc.tile_pool(name="const", bufs=1))
        work = ctx.enter_context(tc.tile_pool(name="work", bufs=3))
        psim = ctx.enter_context(tc.tile_pool(name="psim", bufs=2,
                                              space="PSUM"))
        pacc = ctx.enter_context(tc.tile_pool(name="pacc", bufs=2,
                                              space="PSUM"))

        xnT_sb = const.tile([P, N], bf16, name="xnT_sb")
        xnq_sb = const.tile([P, R], bf16, name="xnq_sb")
        wt_sb = const.tile([P, QC * dspan * nwt], bf16, name="wt_sb")
        wb_sb = const.tile([P, QC * dspan * nwb], bf16, name="wb_sb")
        # col 0 = ones (rowsum), cols 1.. = zeros (keep class rows untouched
        # while matmuls on off-diag tiles still target partition base 0)
        ones_sb = const.tile([P, nwt], bf16, name="ones_sb")
        out_sb = const.tile([nacc, R], f32, name="out_sb")

        nc.vector.memset(ones_sb[:], 0.0)
        nc.vector.memset(ones_sb[:, 0:1], 1.0)

        # Loads: k-loop consumes xnt tiles in rotated order 0..47; queries,
        # weights needed from k=0.
        nc.sync.dma_start(xnq_sb[:], xnq[:])
        nc.sync.dma_start(wt_sb[:], wt[:])
        nc.sync.dma_start(wb_sb[:], wb[:])
        for c in range(4):
            csl = slice(c * (N // 4), (c + 1) * (N // 4))
            nc.sync.dma_start(xnT_sb[:, csl], xnt[:, csl])

        def acc_mms(q, g, acc, st3, sb3):
            for j in range(GRP):
                k = g * GRP + j
                if k < dspan:
                    woff = (q * dspan + k) * nwt
                    boff = (q * dspan + k) * nwb
                    nc.tensor.matmul(acc[0:nwt, :],
                                     wt_sb[:, woff:woff + nwt],
                                     st3[:, j, :],
                                     start=(k == 0), stop=False)
                    nc.tensor.matmul(acc[32:nacc, :],
                                     wb_sb[:, boff:boff + nwb],
                                     sb3[:, j, :],
                                     start=(k == 0),
                                     stop=(k == dspan - 1))
                    if k == dspan - 1:
                        # combo region is final ~3/4 of the chunk early;
                        # drain it now so the end-of-kernel DMA is tiny
                        qsl_ = slice(q * QF, (q + 1) * QF)
                        nc.vector.tensor_copy(out_sb[32:nacc, qsl_],
                                              acc[32:nacc, :])
                        nc.sync.dma_start(out[32:nacc, qsl_],
                                          out_sb[32:nacc, qsl_])
                else:
                    nc.tensor.matmul(acc[0:nwt, :], ones_sb[:],
                                     st3[:, j, :],
                                     start=False, stop=(k == KT - 1))

        # Software pipeline: group g's accumulate matmuls are issued after
        # group g+1's sim matmuls + activation, so the PE streams
        # continuously (stays ramped) while Act runs in parallel.
        pending = None
        accs = []
        for q in range(QC):
            qsl = slice(q * QF, (q + 1) * QF)
            acc = pacc.tile([nacc, QF], f32, tag="acc", name="acc")
            accs.append(acc)
            for g in range(NG):
                sim3 = psim.tile([P, GRP, 512], f32, tag="sim", name="sim3")
                for j in range(GRP):
                    k = g * GRP + j
                    ksl = slice(k * P, (k + 1) * P)
                    nc.tensor.matmul(sim3[:, j, 0:QF], xnT_sb[:, ksl],
                                     xnq_sb[:, qsl], start=True, stop=True)
                diag = g < dgrp
                st3 = work.tile([P, GRP, QF], bf16, tag="st3", name="st3")
                sb3 = None
                if diag:
                    sb3 = work.tile([P, GRP, QF], bf16, tag="sb3", name="sb3")
                    nc.scalar.activation(sb3[:, :, :], sim3[:, :, 0:QF],
                                         EXP, scale=scale_b)
                    if square_mode:
                        nc.vector.tensor_mul(st3[:, :, :], sb3[:, :, :],
                                             sb3[:, :, :])
                    else:
                        nc.scalar.activation(st3[:, :, :], sim3[:, :, 0:QF],
                                             EXP, scale=scale_t)
                else:
                    nc.scalar.activation(st3[:, :, :], sim3[:, :, 0:QF],
                                         EXP, scale=scale_t)
                if pending is not None:
                    acc_mms(*pending)
                pending = (q, g, acc, st3, sb3)
        acc_mms(*pending)
        for q in range(QC):
            qsl = slice(q * QF, (q + 1) * QF)
            nc.vector.tensor_copy(out_sb[0:nwt, qsl], accs[q][0:nwt, :])
            nc.sync.dma_start(out[0:nwt, qsl], out_sb[0:nwt, qsl])

    nc.compile()
    return nc


def _get_compiled(scale_t, scale_b, square_mode, dspan, ncls):
    key = (round(scale_t, 9), round(scale_b, 9), square_mode, dspan, ncls)
    if key not in _compile_cache:
        _compile_cache[key] = _build(scale_t, scale_b, square_mode,
                                     dspan, ncls)
    return _compile_cache[key]


def _round_f32r(v):
    """Round fp32 mantissa to 11 explicit bits (the PE's FP32r format)."""
    b = np.ascontiguousarray(v, dtype=np.float32).view(np.uint32).astype(np.uint64)
    r = ((b + np.uint64(1 << 11)) >> np.uint64(12)) << np.uint64(12)
    return r.astype(np.uint32).view(np.float32)


def _host_prep(input, temperature, targets, batch0):
    import ml_dtypes

    x = np.asarray(input, dtype=np.float32)
    t = float(np.clip(np.float32(temperature), MIN_T, MAX_T))
    scale_t = 1.0 / t
    scale_b = 1.0 / TEMP_BATCH
    square_mode = abs(scale_t - 2.0 * scale_b) < 1e-6

    tg0 = np.asarray(targets).astype(np.int64)
    bt0 = np.asarray(batch0).astype(np.int64)
    perm = np.lexsort((bt0, tg0))
    tg = tg0[perm]
    bt = bt0[perm]
    x = x[perm]

    norms = np.sqrt((x * x).sum(axis=1, keepdims=True, dtype=np.float32))
    norms = np.maximum(norms, np.float32(EPS)).astype(np.float32)
    xn = (x / norms).astype(ml_dtypes.bfloat16)         # device matmul operand
    s_ii = (xn.astype(np.float32) ** 2).sum(axis=1, dtype=np.float32)

    # class start/end in sorted order
    cls_start = np.searchsorted(tg, np.arange(NT), side="left")
    cls_end = np.searchsorted(tg, np.arange(NT), side="right")

    # per-core rotation + diag span
    rots, spans = [], []
    for c in range(NCORES):
        lo_cls = tg[c * R]
        hi_cls = tg[(c + 1) * R - 1]
        d0 = int(cls_start[lo_cls]) // P
        d1 = -(-int(cls_end[hi_cls]) // P)
        rots.append(d0)
        spans.append(d1 - d0)
    dspan = -(-max(spans) // GRP) * GRP
    assert dspan <= KT // 2, f"diag span {dspan} too large"

    # per-chunk class slots
    ncls = 1
    chunk_cls = []          # list over (core, chunk) of class id list
    for c in range(NCORES):
        for q in range(QC):
            seg = tg[c * R + q * QF: c * R + (q + 1) * QF]
            cl = sorted(set(seg.tolist()))
            chunk_cls.append(cl)
            ncls = max(ncls, len(cl))

    # per-core device inputs (bf16): xnT in SBUF layout [128(d), N(keys)]
    xnq_all = np.ascontiguousarray(xn.T)                # [128, N]

    nwt = 1 + ncls
    nwb = 5 * ncls
    in_maps = []
    for c in range(NCORES):
        rot = rots[c]
        # key columns in rotated tile order, already transposed [d, keys]
        colord = ((rot * P + np.arange(N)) % N)
        xnt_c = np.ascontiguousarray(xnq_all[:, colord])
        wt_c = np.zeros((P, QC * dspan * nwt), dtype=np.float32)
        wb_c = np.zeros((P, QC * dspan * nwb), dtype=np.float32)
        for q in range(QC):
            cl = chunk_cls[c * QC + q]
            for k in range(dspan):
                gt = (rot + k) % KT
                ktg = tg[gt * P:(gt + 1) * P]
                kbt = bt[gt * P:(gt + 1) * P]
                woff = (q * dspan + k) * nwt
                boff = (q * dspan + k) * nwb
                wt_c[:, woff] = 1.0
                for si, cid in enumerate(cl):
                    m = ktg == cid
                    wt_c[m, woff + 1 + si] = 1.0
                    for b in range(NB):
                        wb_c[m & (kbt == b), boff + si * 5 + b] = 1.0
        in_maps.append({
            "xnt": xnt_c,
            "xnq": np.ascontiguousarray(xnq_all[:, c * R:(c + 1) * R]),
            "wt": wt_c.astype(ml_dtypes.bfloat16),
            "wb": wb_c.astype(ml_dtypes.bfloat16),
        })

    meta = dict(tg=tg, bt=bt, s_ii=s_ii, chunk_cls=chunk_cls, ncls=ncls,
                dspan=dspan, scale_t=scale_t, scale_b=scale_b,
                square_mode=square_mode)
    return in_maps, meta


def _epilogue(acc, meta, weight_target, weight_batch0):
    """acc: [NCORES, nacc, R] device slot sums. Everything here is O(N)."""
    f = np.float64
    tg, bt, s_ii = meta["tg"], meta["bt"], meta["s_ii"]
    ncls = meta["ncls"]
    nacc = 32 + 5 * ncls      # combo region at partition base 32
    scale_t, scale_b = meta["scale_t"], meta["scale_b"]

    nchunk = NCORES * QC
    ch = acc.reshape(NCORES, nacc, QC, QF).transpose(
        0, 2, 1, 3).reshape(nchunk, nacc, QF)

    # slot row per sorted query
    ci = np.repeat(np.arange(nchunk), QF)           # chunk index
    col = np.tile(np.arange(QF), nchunk)
    si = np.zeros(N, dtype=np.int64)
    for ic in range(nchunk):
        cl = meta["chunk_cls"][ic]
        lut = np.full(NT, -1, dtype=np.int64)
        for s, cid in enumerate(cl):
            lut[cid] = s
        seg = slice(ic * QF, (ic + 1) * QF)
        si[seg] = lut[tg[seg]]
    assert (si >= 0).all()

    rowsum_t = ch[ci, 0, col].astype(f)
    possum_t = ch[ci, 1 + si, col].astype(f)                    # incl self
    own_tb = ch[ci, 32 + si * 5 + bt, col].astype(f)            # incl self
    samet_b = ch[ci][np.arange(N)[:, None],
                     (32 + si * 5)[:, None] + np.arange(5)[None, :],
                     col[:, None]].sum(axis=1, dtype=f)

    diag_t = np.exp(scale_t * s_ii.astype(f))
    diag_b = np.exp(scale_b * s_ii.astype(f))

    cnt_t = np.bincount(tg, minlength=NT)
    n_tb = np.zeros((NT, NB), dtype=np.int64)
    np.add.at(n_tb, (tg, bt), 1)

    # ---- target SNN loss ----
    pos_t = possum_t - diag_t
    neg_t = rowsum_t - possum_t
    cnt_pos = cnt_t[tg]
    cnt_neg = N - cnt_pos
    valid_t = (cnt_pos >= 2) & (cnt_neg >= 1)
    pos_s = np.where(valid_t, pos_t, 1.0)
    neg_s = np.where(valid_t, neg_t, 1.0)
    loss_i = -np.log(pos_s / (pos_s + neg_s))
    lsum = np.bincount(tg, weights=np.where(valid_t, loss_i, 0.0), minlength=NT)
    vcnt = np.bincount(tg, weights=valid_t.astype(f), minlength=NT)
    mean = lsum / np.maximum(vcnt, 1.0)
    wt_w = np.asarray(weight_target).astype(f)
    loss_target = np.where(vcnt > 0, mean * wt_w, 0.0).sum()

    # ---- batch-effect loss ----
    pos_b = own_tb - diag_b
    neg_b = samet_b - own_tb
    cnt_pos_b = n_tb[tg, bt]
    cnt_neg_b = cnt_t[tg] - cnt_pos_b
    valid_b = (cnt_pos_b >= 2) & (cnt_neg_b >= 1)
    pos_bs = np.where(valid_b, pos_b, 1.0)
    neg_bs = np.where(valid_b, neg_b, 1.0)
    loss_bi = -np.log(pos_bs / (pos_bs + neg_bs))
    inv = np.where(valid_b, 1.0 / np.where(valid_b, loss_bi, 1.0), 0.0)
    lsum_b = np.bincount(bt, weights=inv, minlength=NB)
    vcnt_b = np.bincount(bt, weights=valid_b.astype(f), minlength=NB)
    mean_b = lsum_b / np.maximum(vcnt_b, 1.0)
    wb_w = np.asarray(weight_batch0).astype(f)
    loss_batch = np.where(vcnt_b > 0, mean_b * wb_w, 0.0).sum()

    return np.float32(0.9 * loss_target + 0.1 * loss_batch)


def _run_with_retry(nc, in_maps, core_ids, attempts=3):
    import time as _time

    from concourse.bass_utils import run_bass_kernel_spmd

    for i in range(attempts):
        try:
            return run_bass_kernel_spmd(nc, in_maps, core_ids)
        except Exception:
            if i == attempts - 1:
                raise
            _time.sleep(90)  # transient NRT device errors clear after a pause


def kernel(input, temperature, weight_target, weight_batch0, targets, batch0):
    global LAST_RESULT

    in_maps, meta = _host_prep(input, temperature, targets, batch0)
    nc = _get_compiled(meta["scale_t"], meta["scale_b"], meta["square_mode"],
                       meta["dspan"], meta["ncls"])

    LAST_RESULT = _run_with_retry(nc, in_maps, list(range(NCORES)))
    acc = np.stack([LAST_RESULT.results[c]["out"] for c in range(NCORES)])

    return _epilogue(acc, meta, weight_target, weight_batch0)


# revision 30
# speedup vs baseline: 1.1856x; 1.0101x over previous
"""Trainium2 Bass kernel for CustomSNNLoss (nn_CustomSNNLoss_36429912604816).

Strategy v2 (sorted-class block-diagonal decomposition):
  - Host: sort rows by (target, batch) so classes/combos are contiguous key
    ranges. Normalize x, pre-transpose to xnT [D=128, N].
  - Each of 8 cores owns R = 768 query rows (contiguous in sorted order) and
    all 6144 keys. Key tiles are ROTATED per core so the core's block-diagonal
    key range (keys sharing a target class with the core's queries) always
    sits at tiles [0, DSPAN) -- one uniform SPMD program, per-core data.
  - Per (chunk of 384 queries) x (group of 3 key tiles):
      sim  = xnT[ktile].T @ xnq          (f32r matmul -> PSUM, 3 banks/group)
      one Act instr: St = exp(sim/t) (bf16) spanning all 3 banks
        (diag groups: Sb = exp(2*sim), DVE squares -> St when t == 0.25)
      PE: rowsum via ones-weights matmul on every tile; class-slot (St) and
          combo-slot (Sb) one-hot matmuls on the DSPAN diag tiles only.
    Accumulators live in one PSUM bank: rows [0:1+NCLS] = ones+class slots
    (St), rows [1+NCLS:] = combo slots (Sb).
  - Host epilogue (O(N)): decode slots, per-row pos/neg sums, -log losses,
    validity masks, class-weighted means, final scalar.
"""

import numpy as np

N, D = 6144, 128
P = 128                 # partitions / contraction tile
NCORES = 8
R = N // NCORES         # 768 query rows per core
KT = N // P             # 48 key tiles
QC = 2                  # query chunks per core
QF = R // QC            # 384 free-dim per matmul
GRP = 3                 # key tiles per PSUM/Act group
NG = KT // GRP          # 16 groups per chunk
NT, NB = 20, 5          # target classes, batch keys
MIN_T, MAX_T = 0.1, 1.0
TEMP_BATCH = 0.5
EPS = 1e-8

_compile_cache = {}
LAST_RESULT = None  # BassKernelResults from the most recent device run


def _build(scale_t: float, scale_b: float, square_mode: bool,
           dspan: int, ncls: int):
    """dspan: #rotated key tiles with class/combo matmuls (multiple of GRP).
    ncls: number of class slots per chunk. Acc rows = 1 + ncls + 5*ncls."""
    from contextlib import ExitStack

    import concourse.bacc as bacc
    import concourse.mybir as mybir
    import concourse.tile as tile

    f32 = mybir.dt.float32
    bf16 = mybir.dt.bfloat16
    EXP = mybir.ActivationFunctionType.Exp

    nwt = 1 + ncls            # ones col + class slots
    nwb = 5 * ncls            # combo slots
    nacc = 32 + nwb           # combo region lives at partition base 32
    dgrp = dspan // GRP       # diag groups per chunk

    nc = bacc.Bacc("TRN2", target_bir_lowering=False, debug=False,
                   enable_asserts=False)

    # xin = [xnq (R cols) | xnT (N cols)]; ww = [wt | wb] per (chunk, tile)
    nww = QC * dspan * (nwt + nwb)
    xin = nc.dram_tensor("xin", [P, R + N], bf16, kind="ExternalInput").ap()
    ww = nc.dram_tensor("ww", [P, nww], bf16, kind="ExternalInput").ap()
    out = nc.dram_tensor("out", [nacc, R], f32, kind="ExternalOutput").ap()

    with tile.TileContext(nc) as tc, ExitStack() as ctx:
        const = ctx.enter_context(tc.tile_pool(name="const", bufs=1))
        work = ctx.enter_context(tc.tile_pool(name="work", bufs=3))
        psim = ctx.enter_context(tc.tile_pool(name="psim", bufs=2,
                                              space="PSUM"))
        pacc = ctx.enter_context(tc.tile_pool(name="pacc", bufs=2,
                                              space="PSUM"))

        xin_sb = const.tile([P, R + N], bf16, name="xin_sb")
        ww_sb = const.tile([P, nww], bf16, name="ww_sb")
        # col 0 = ones (rowsum), cols 1.. = zeros (keep class rows untouched
        # while matmuls on off-diag tiles still target partition base 0)
        ones_sb = const.tile([P, nwt], bf16, name="ones_sb")
        out_sb = const.tile([nacc, R], f32, name="out_sb")

        nc.vector.memset(ones_sb[:], 0.0)
        nc.vector.memset(ones_sb[:, 0:1], 1.0)

        # Loads: k-loop consumes xnt tiles in rotated order 0..47; queries,
        # weights needed from k=0.
        nc.sync.dma_start(ww_sb[:], ww[:])
        for lo, hi in ((0, R + 1536), (R + 1536, R + 3584),
                       (R + 3584, R + N)):
            nc.sync.dma_start(xin_sb[:, lo:hi], xin[:, lo:hi])

        def acc_mms(q, g, acc, st3, sb3):
            for j in range(GRP):
                k = g * GRP + j
                if k < dspan:
                    off = (q * dspan + k) * (nwt + nwb)
                    nc.tensor.matmul(acc[0:nwt, :],
                                     ww_sb[:, off:off + nwt],
                                     st3[:, j, :],
                                     start=(k == 0), stop=False)
                    nc.tensor.matmul(acc[32:nacc, :],
                                     ww_sb[:, off + nwt:off + nwt + nwb],
                                     sb3[:, j, :],
                                     start=(k == 0),
                                     stop=(k == dspan - 1))
                    if k == dspan - 1:
                        # combo region is final ~3/4 of the chunk early;
                        # drain it now so the end-of-kernel DMA is tiny
                        qsl_ = slice(q * QF, (q + 1) * QF)
                        nc.vector.tensor_copy(out_sb[32:nacc, qsl_],
                                              acc[32:nacc, :])
                        nc.sync.dma_start(out[32:nacc, qsl_],
                                          out_sb[32:nacc, qsl_])
                else:
                    nc.tensor.matmul(acc[0:nwt, :], ones_sb[:],
                                     st3[:, j, :],
                                     start=False, stop=(k == KT - 1))

        # Software pipeline: group g's accumulate matmuls are issued after
        # group g+1's sim matmuls + activation, so the PE streams
        # continuously (stays ramped) while Act runs in parallel.
        pending = None
        accs = []
        for q in range(QC):
            qsl = slice(q * QF, (q + 1) * QF)
            acc = pacc.tile([nacc, QF], f32, tag="acc", name="acc")
            accs.append(acc)
            for g in range(NG):
                sim3 = psim.tile([P, GRP, 512], f32, tag="sim", name="sim3")
                for j in range(GRP):
                    k = g * GRP + j
                    ksl = slice(R + k * P, R + (k + 1) * P)
                    nc.tensor.matmul(sim3[:, j, 0:QF], xin_sb[:, ksl],
                                     xin_sb[:, qsl], start=True, stop=True)
                diag = g < dgrp
                st3 = work.tile([P, GRP, QF], bf16, tag="st3", name="st3")
                sb3 = None
                if diag:
                    sb3 = work.tile([P, GRP, QF], bf16, tag="sb3", name="sb3")
                    nc.scalar.activation(sb3[:, :, :], sim3[:, :, 0:QF],
                                         EXP, scale=scale_b)
                    if square_mode:
                        nc.vector.tensor_mul(st3[:, :, :], sb3[:, :, :],
                                             sb3[:, :, :])
                    else:
                        nc.scalar.activation(st3[:, :, :], sim3[:, :, 0:QF],
                                             EXP, scale=scale_t)
                else:
                    nc.scalar.activation(st3[:, :, :], sim3[:, :, 0:QF],
                                         EXP, scale=scale_t)
                if pending is not None:
                    acc_mms(*pending)
                pending = (q, g, acc, st3, sb3)
        acc_mms(*pending)
        for q in range(QC):
            qsl = slice(q * QF, (q + 1) * QF)
            nc.vector.tensor_copy(out_sb[0:nwt, qsl], accs[q][0:nwt, :])
            nc.sync.dma_start(out[0:nwt, qsl], out_sb[0:nwt, qsl])

    nc.compile()
    return nc


def _get_compiled(scale_t, scale_b, square_mode, dspan, ncls):
    key = (round(scale_t, 9), round(scale_b, 9), square_mode, dspan, ncls)
    if key not in _compile_cache:
        _compile_cache[key] = _build(scale_t, scale_b, square_mode,
                                     dspan, ncls)
    return _compile_cache[key]


def _round_f32r(v):
    """Round fp32 mantissa to 11 explicit bits (the PE's FP32r format)."""
    b = np.ascontiguousarray(v, dtype=np.float32).view(np.uint32).astype(np.uint64)
    r = ((b + np.uint64(1 << 11)) >> np.uint64(12)) << np.uint64(12)
    return r.astype(np.uint32).view(np.float32)


def _host_prep(input, temperature, targets, batch0):
    import ml_dtypes

    x = np.asarray(input, dtype=np.float32)
    t = float(np.clip(np.float32(temperature), MIN_T, MAX_T))
    scale_t = 1.0 / t
    scale_b = 1.0 / TEMP_BATCH
    square_mode = abs(scale_t - 2.0 * scale_b) < 1e-6

    tg0 = np.asarray(targets).astype(np.int64)
    bt0 = np.asarray(batch0).astype(np.int64)
    perm = np.lexsort((bt0, tg0))
    tg = tg0[perm]
    bt = bt0[perm]
    x = x[perm]

    norms = np.sqrt((x * x).sum(axis=1, keepdims=True, dtype=np.float32))
    norms = np.maximum(norms, np.float32(EPS)).astype(np.float32)
    xn = (x / norms).astype(ml_dtypes.bfloat16)         # device matmul operand
    s_ii = (xn.astype(np.float32) ** 2).sum(axis=1, dtype=np.float32)

    # class start/end in sorted order
    cls_start = np.searchsorted(tg, np.arange(NT), side="left")
    cls_end = np.searchsorted(tg, np.arange(NT), side="right")

    # per-core rotation + diag span
    rots, spans = [], []
    for c in range(NCORES):
        lo_cls = tg[c * R]
        hi_cls = tg[(c + 1) * R - 1]
        d0 = int(cls_start[lo_cls]) // P
        d1 = -(-int(cls_end[hi_cls]) // P)
        rots.append(d0)
        spans.append(d1 - d0)
    dspan = -(-max(spans) // GRP) * GRP
    assert dspan <= KT // 2, f"diag span {dspan} too large"

    # per-chunk class slots
    ncls = 1
    chunk_cls = []          # list over (core, chunk) of class id list
    for c in range(NCORES):
        for q in range(QC):
            seg = tg[c * R + q * QF: c * R + (q + 1) * QF]
            cl = sorted(set(seg.tolist()))
            chunk_cls.append(cl)
            ncls = max(ncls, len(cl))

    # per-core device inputs (bf16): xnT in SBUF layout [128(d), N(keys)]
    xnq_all = np.ascontiguousarray(xn.T)                # [128, N]

    nwt = 1 + ncls
    nwb = 5 * ncls
    in_maps = []
    nws = nwt + nwb
    for c in range(NCORES):
        rot = rots[c]
        # key columns in rotated tile order, already transposed [d, keys]
        colord = ((rot * P + np.arange(N)) % N)
        xin_c = np.concatenate(
            [xnq_all[:, c * R:(c + 1) * R], xnq_all[:, colord]], axis=1)
        ww_c = np.zeros((P, QC * dspan * nws), dtype=np.float32)
        for q in range(QC):
            cl = chunk_cls[c * QC + q]
            for k in range(dspan):
                gt = (rot + k) % KT
                ktg = tg[gt * P:(gt + 1) * P]
                kbt = bt[gt * P:(gt + 1) * P]
                off = (q * dspan + k) * nws
                ww_c[:, off] = 1.0
                for si, cid in enumerate(cl):
                    m = ktg == cid
                    ww_c[m, off + 1 + si] = 1.0
                    for b in range(NB):
                        ww_c[m & (kbt == b), off + nwt + si * 5 + b] = 1.0
        in_maps.append({
            "xin": np.ascontiguousarray(xin_c),
            "ww": ww_c.astype(ml_dtypes.bfloat16),
        })

    meta = dict(tg=tg, bt=bt, s_ii=s_ii, chunk_cls=chunk_cls, ncls=ncls,
                dspan=dspan, scale_t=scale_t, scale_b=scale_b,
                square_mode=square_mode)
    return in_maps, meta


def _epilogue(acc, meta, weight_target, weight_batch0):
    """acc: [NCORES, nacc, R] device slot sums. Everything here is O(N)."""
    f = np.float64
    tg, bt, s_ii = meta["tg"], meta["bt"], meta["s_ii"]
    ncls = meta["ncls"]
    nacc = 32 + 5 * ncls      # combo region at partition base 32
    scale_t, scale_b = meta["scale_t"], meta["scale_b"]

    nchunk = NCORES * QC
    ch = acc.reshape(NCORES, nacc, QC, QF).transpose(
        0, 2, 1, 3).reshape(nchunk, nacc, QF)

    # slot row per sorted query
    ci = np.repeat(np.arange(nchunk), QF)           # chunk index
    col = np.tile(np.arange(QF), nchunk)
    si = np.zeros(N, dtype=np.int64)
    for ic in range(nchunk):
        cl = meta["chunk_cls"][ic]
        lut = np.full(NT, -1, dtype=np.int64)
        for s, cid in enumerate(cl):
            lut[cid] = s
        seg = slice(ic * QF, (ic + 1) * QF)
        si[seg] = lut[tg[seg]]
    assert (si >= 0).all()

    rowsum_t = ch[ci, 0, col].astype(f)
    possum_t = ch[ci, 1 + si, col].astype(f)                    # incl self
    own_tb = ch[ci, 32 + si * 5 + bt, col].astype(f)            # incl self
    samet_b = ch[ci][np.arange(N)[:, None],
                     (32 + si * 5)[:, None] + np.arange(5)[None, :],
                     col[:, None]].sum(axis=1, dtype=f)

    diag_t = np.exp(scale_t * s_ii.astype(f))
    diag_b = np.exp(scale_b * s_ii.astype(f))

    cnt_t = np.bincount(tg, minlength=NT)
    n_tb = np.zeros((NT, NB), dtype=np.int64)
    np.add.at(n_tb, (tg, bt), 1)

    # ---- target SNN loss ----
    pos_t = possum_t - diag_t
    neg_t = rowsum_t - possum_t
    cnt_pos = cnt_t[tg]
    cnt_neg = N - cnt_pos
    valid_t = (cnt_pos >= 2) & (cnt_neg >= 1)
    pos_s = np.where(valid_t, pos_t, 1.0)
    neg_s = np.where(valid_t, neg_t, 1.0)
    loss_i = -np.log(pos_s / (pos_s + neg_s))
    lsum = np.bincount(tg, weights=np.where(valid_t, loss_i, 0.0), minlength=NT)
    vcnt = np.bincount(tg, weights=valid_t.astype(f), minlength=NT)
    mean = lsum / np.maximum(vcnt, 1.0)
    wt_w = np.asarray(weight_target).astype(f)
    loss_target = np.where(vcnt > 0, mean * wt_w, 0.0).sum()

    # ---- batch-effect loss ----
    pos_b = own_tb - diag_b
    neg_b = samet_b - own_tb
    cnt_pos_b = n_tb[tg, bt]
    cnt_neg_b = cnt_t[tg] - cnt_pos_b
    valid_b = (cnt_pos_b >= 2) & (cnt_neg_b >= 1)
    pos_bs = np.where(valid_b, pos_b, 1.0)
    neg_bs = np.where(valid_b, neg_b, 1.0)
    loss_bi = -np.log(pos_bs / (pos_bs + neg_bs))
    inv = np.where(valid_b, 1.0 / np.where(valid_b, loss_bi, 1.0), 0.0)
    lsum_b = np.bincount(bt, weights=inv, minlength=NB)
    vcnt_b = np.bincount(bt, weights=valid_b.astype(f), minlength=NB)
    mean_b = lsum_b / np.maximum(vcnt_b, 1.0)
    wb_w = np.asarray(weight_batch0).astype(f)
    loss_batch = np.where(vcnt_b > 0, mean_b * wb_w, 0.0).sum()

    return np.float32(0.9 * loss_target + 0.1 * loss_batch)


def _run_with_retry(nc, in_maps, core_ids, attempts=3):
    import time as _time

    from concourse.bass_utils import run_bass_kernel_spmd

    for i in range(attempts):
        try:
            return run_bass_kernel_spmd(nc, in_maps, core_ids)
        except Exception:
            if i == attempts - 1:
                raise
            _time.sleep(90)  # transient NRT device errors clear after a pause


def kernel(input, temperature, weight_target, weight_batch0, targets, batch0):
    global LAST_RESULT

    in_maps, meta = _host_prep(input, temperature, targets, batch0)
    nc = _get_compiled(meta["scale_t"], meta["scale_b"], meta["square_mode"],
                       meta["dspan"], meta["ncls"])

    LAST_RESULT = _run_with_retry(nc, in_maps, list(range(NCORES)))
    acc = np.stack([LAST_RESULT.results[c]["out"] for c in range(NCORES)])

    return _epilogue(acc, meta, weight_target, weight_batch0)


# revision 31
# speedup vs baseline: 1.2312x; 1.0385x over previous
"""Trainium2 Bass kernel for CustomSNNLoss (nn_CustomSNNLoss_36429912604816).

Strategy v2 (sorted-class block-diagonal decomposition):
  - Host: sort rows by (target, batch) so classes/combos are contiguous key
    ranges. Normalize x, pre-transpose to xnT [D=128, N].
  - Each of 8 cores owns R = 768 query rows (contiguous in sorted order) and
    all 6144 keys. Key tiles are ROTATED per core so the core's block-diagonal
    key range (keys sharing a target class with the core's queries) always
    sits at tiles [0, DSPAN) -- one uniform SPMD program, per-core data.
  - Per (chunk of 384 queries) x (group of 3 key tiles):
      sim  = xnT[ktile].T @ xnq          (f32r matmul -> PSUM, 3 banks/group)
      one Act instr: St = exp(sim/t) (bf16) spanning all 3 banks
        (diag groups: Sb = exp(2*sim), DVE squares -> St when t == 0.25)
      PE: rowsum via ones-weights matmul on every tile; class-slot (St) and
          combo-slot (Sb) one-hot matmuls on the DSPAN diag tiles only.
    Accumulators live in one PSUM bank: rows [0:1+NCLS] = ones+class slots
    (St), rows [1+NCLS:] = combo slots (Sb).
  - Host epilogue (O(N)): decode slots, per-row pos/neg sums, -log losses,
    validity masks, class-weighted means, final scalar.
"""

import numpy as np

N, D = 6144, 128
P = 128                 # partitions / contraction tile
NCORES = 8
R = N // NCORES         # 768 query rows per core
KT = N // P             # 48 key tiles
QC = 2                  # query chunks per core
QF = R // QC            # 384 free-dim per matmul
GRP = 3                 # key tiles per PSUM/Act group
NG = KT // GRP          # 16 groups per chunk
NT, NB = 20, 5          # target classes, batch keys
MIN_T, MAX_T = 0.1, 1.0
TEMP_BATCH = 0.5
EPS = 1e-8

_compile_cache = {}
LAST_RESULT = None  # BassKernelResults from the most recent device run


def _build(scale_t: float, scale_b: float, square_mode: bool,
           dspan: int, ncls: int):
    """dspan: #rotated key tiles with class/combo matmuls (multiple of GRP).
    ncls: number of class slots per chunk. Acc rows = 1 + ncls + 5*ncls."""
    from contextlib import ExitStack

    import concourse.bacc as bacc
    import concourse.mybir as mybir
    import concourse.tile as tile

    f32 = mybir.dt.float32
    bf16 = mybir.dt.bfloat16
    EXP = mybir.ActivationFunctionType.Exp

    nwt = 1 + ncls            # ones col + class slots
    nwb = 5 * ncls            # combo slots
    nacc = 32 + nwb           # combo region lives at partition base 32
    dgrp = dspan // GRP       # diag groups per chunk

    nc = bacc.Bacc("TRN2", target_bir_lowering=False, debug=False,
                   enable_asserts=False)

    # xin = [xnq (R cols) | xnT (N cols)]; ww = [wt | wb] per (chunk, tile)
    nww = QC * dspan * (nwt + nwb)
    xin = nc.dram_tensor("xin", [P, R + N], bf16, kind="ExternalInput").ap()
    ww = nc.dram_tensor("ww", [P, nww], bf16, kind="ExternalInput").ap()
    out = nc.dram_tensor("out", [nacc, R], f32, kind="ExternalOutput").ap()

    with tile.TileContext(nc) as tc, ExitStack() as ctx:
        const = ctx.enter_context(tc.tile_pool(name="const", bufs=1))
        work = ctx.enter_context(tc.tile_pool(name="work", bufs=5))
        psim = ctx.enter_context(tc.tile_pool(name="psim", bufs=2,
                                              space="PSUM"))
        pacc = ctx.enter_context(tc.tile_pool(name="pacc", bufs=2,
                                              space="PSUM"))

        xin_sb = const.tile([P, R + N], bf16, name="xin_sb")
        ww_sb = const.tile([P, nww], bf16, name="ww_sb")
        # col 0 = ones (rowsum), cols 1.. = zeros (keep class rows untouched
        # while matmuls on off-diag tiles still target partition base 0)
        ones_sb = const.tile([P, nwt], bf16, name="ones_sb")
        out_sb = const.tile([nacc, R], f32, name="out_sb")

        nc.vector.memset(ones_sb[:], 0.0)
        nc.vector.memset(ones_sb[:, 0:1], 1.0)

        # Loads: k-loop consumes xnt tiles in rotated order 0..47; queries,
        # weights needed from k=0.
        nc.sync.dma_start(ww_sb[:], ww[:])
        for lo, hi in ((0, R + 1536), (R + 1536, R + 3584),
                       (R + 3584, R + N)):
            nc.sync.dma_start(xin_sb[:, lo:hi], xin[:, lo:hi])

        def acc_mms(q, g, acc, st3, sb3):
            for j in range(GRP):
                k = g * GRP + j
                if k < dspan:
                    off = (q * dspan + k) * (nwt + nwb)
                    nc.tensor.matmul(acc[0:nwt, :],
                                     ww_sb[:, off:off + nwt],
                                     st3[:, j, :],
                                     start=(k == 0), stop=False)
                    nc.tensor.matmul(acc[32:nacc, :],
                                     ww_sb[:, off + nwt:off + nwt + nwb],
                                     sb3[:, j, :],
                                     start=(k == 0),
                                     stop=(k == dspan - 1))
                    if k == dspan - 1:
                        # combo region is final ~3/4 of the chunk early;
                        # drain it now so the end-of-kernel DMA is tiny
                        qsl_ = slice(q * QF, (q + 1) * QF)
                        nc.vector.tensor_copy(out_sb[32:nacc, qsl_],
                                              acc[32:nacc, :])
                        nc.sync.dma_start(out[32:nacc, qsl_],
                                          out_sb[32:nacc, qsl_])
                else:
                    nc.tensor.matmul(acc[0:nwt, :], ones_sb[:],
                                     st3[:, j, :],
                                     start=False, stop=(k == KT - 1))

        # Software pipeline: group g's accumulate matmuls are issued after
        # group g+1's sim matmuls + activation, so the PE streams
        # continuously (stays ramped) while Act runs in parallel.
        pending = None
        accs = []
        for q in range(QC):
            qsl = slice(q * QF, (q + 1) * QF)
            acc = pacc.tile([nacc, QF], f32, tag="acc", name="acc")
            accs.append(acc)
            for g in range(NG):
                sim3 = psim.tile([P, GRP, 512], f32, tag="sim", name="sim3")
                for j in range(GRP):
                    k = g * GRP + j
                    ksl = slice(R + k * P, R + (k + 1) * P)
                    nc.tensor.matmul(sim3[:, j, 0:QF], xin_sb[:, ksl],
                                     xin_sb[:, qsl], start=True, stop=True)
                diag = g < dgrp
                st3 = work.tile([P, GRP, QF], bf16, tag="st3", name="st3")
                sb3 = None
                if diag:
                    sb3 = work.tile([P, GRP, QF], bf16, tag="sb3", name="sb3")
                    nc.scalar.activation(sb3[:, :, :], sim3[:, :, 0:QF],
                                         EXP, scale=scale_b)
                    if square_mode:
                        nc.vector.tensor_mul(st3[:, :, :], sb3[:, :, :],
                                             sb3[:, :, :])
                    else:
                        nc.scalar.activation(st3[:, :, :], sim3[:, :, 0:QF],
                                             EXP, scale=scale_t)
                else:
                    nc.scalar.activation(st3[:, :, :], sim3[:, :, 0:QF],
                                         EXP, scale=scale_t)
                if pending is not None:
                    acc_mms(*pending)
                pending = (q, g, acc, st3, sb3)
        acc_mms(*pending)
        for q in range(QC):
            qsl = slice(q * QF, (q + 1) * QF)
            nc.vector.tensor_copy(out_sb[0:nwt, qsl], accs[q][0:nwt, :])
            nc.sync.dma_start(out[0:nwt, qsl], out_sb[0:nwt, qsl])

    nc.compile()
    return nc


def _get_compiled(scale_t, scale_b, square_mode, dspan, ncls):
    key = (round(scale_t, 9), round(scale_b, 9), square_mode, dspan, ncls)
    if key not in _compile_cache:
        _compile_cache[key] = _build(scale_t, scale_b, square_mode,
                                     dspan, ncls)
    return _compile_cache[key]


def _round_f32r(v):
    """Round fp32 mantissa to 11 explicit bits (the PE's FP32r format)."""
    b = np.ascontiguousarray(v, dtype=np.float32).view(np.uint32).astype(np.uint64)
    r = ((b + np.uint64(1 << 11)) >> np.uint64(12)) << np.uint64(12)
    return r.astype(np.uint32).view(np.float32)


def _host_prep(input, temperature, targets, batch0):
    import ml_dtypes

    x = np.asarray(input, dtype=np.float32)
    t = float(np.clip(np.float32(temperature), MIN_T, MAX_T))
    scale_t = 1.0 / t
    scale_b = 1.0 / TEMP_BATCH
    square_mode = abs(scale_t - 2.0 * scale_b) < 1e-6

    tg0 = np.asarray(targets).astype(np.int64)
    bt0 = np.asarray(batch0).astype(np.int64)
    perm = np.lexsort((bt0, tg0))
    tg = tg0[perm]
    bt = bt0[perm]
    x = x[perm]

    norms = np.sqrt((x * x).sum(axis=1, keepdims=True, dtype=np.float32))
    norms = np.maximum(norms, np.float32(EPS)).astype(np.float32)
    xn = (x / norms).astype(ml_dtypes.bfloat16)         # device matmul operand
    s_ii = (xn.astype(np.float32) ** 2).sum(axis=1, dtype=np.float32)

    # class start/end in sorted order
    cls_start = np.searchsorted(tg, np.arange(NT), side="left")
    cls_end = np.searchsorted(tg, np.arange(NT), side="right")

    # per-core rotation + diag span
    rots, spans = [], []
    for c in range(NCORES):
        lo_cls = tg[c * R]
        hi_cls = tg[(c + 1) * R - 1]
        d0 = int(cls_start[lo_cls]) // P
        d1 = -(-int(cls_end[hi_cls]) // P)
        rots.append(d0)
        spans.append(d1 - d0)
    dspan = -(-max(spans) // GRP) * GRP
    assert dspan <= KT // 2, f"diag span {dspan} too large"

    # per-chunk class slots
    ncls = 1
    chunk_cls = []          # list over (core, chunk) of class id list
    for c in range(NCORES):
        for q in range(QC):
            seg = tg[c * R + q * QF: c * R + (q + 1) * QF]
            cl = sorted(set(seg.tolist()))
            chunk_cls.append(cl)
            ncls = max(ncls, len(cl))

    # per-core device inputs (bf16): xnT in SBUF layout [128(d), N(keys)]
    xnq_all = np.ascontiguousarray(xn.T)                # [128, N]

    nwt = 1 + ncls
    nwb = 5 * ncls
    in_maps = []
    nws = nwt + nwb
    for c in range(NCORES):
        rot = rots[c]
        # key columns in rotated tile order, already transposed [d, keys]
        colord = ((rot * P + np.arange(N)) % N)
        xin_c = np.concatenate(
            [xnq_all[:, c * R:(c + 1) * R], xnq_all[:, colord]], axis=1)
        ww_c = np.zeros((P, QC * dspan * nws), dtype=np.float32)
        for q in range(QC):
            cl = chunk_cls[c * QC + q]
            for k in range(dspan):
                gt = (rot + k) % KT
                ktg = tg[gt * P:(gt + 1) * P]
                kbt = bt[gt * P:(gt + 1) * P]
                off = (q * dspan + k) * nws
                ww_c[:, off] = 1.0
                for si, cid in enumerate(cl):
                    m = ktg == cid
                    ww_c[m, off + 1 + si] = 1.0
                    for b in range(NB):
                        ww_c[m & (kbt == b), off + nwt + si * 5 + b] = 1.0
        in_maps.append({
            "xin": np.ascontiguousarray(xin_c),
            "ww": ww_c.astype(ml_dtypes.bfloat16),
        })

    meta = dict(tg=tg, bt=bt, s_ii=s_ii, chunk_cls=chunk_cls, ncls=ncls,
                dspan=dspan, scale_t=scale_t, scale_b=scale_b,
                square_mode=square_mode)
    return in_maps, meta


def _epilogue(acc, meta, weight_target, weight_batch0):
    """acc: [NCORES, nacc, R] device slot sums. Everything here is O(N)."""
    f = np.float64
    tg, bt, s_ii = meta["tg"], meta["bt"], meta["s_ii"]
    ncls = meta["ncls"]
    nacc = 32 + 5 * ncls      # combo region at partition base 32
    scale_t, scale_b = meta["scale_t"], meta["scale_b"]

    nchunk = NCORES * QC
    ch = acc.reshape(NCORES, nacc, QC, QF).transpose(
        0, 2, 1, 3).reshape(nchunk, nacc, QF)

    # slot row per sorted query
    ci = np.repeat(np.arange(nchunk), QF)           # chunk index
    col = np.tile(np.arange(QF), nchunk)
    si = np.zeros(N, dtype=np.int64)
    for ic in range(nchunk):
        cl = meta["chunk_cls"][ic]
        lut = np.full(NT, -1, dtype=np.int64)
        for s, cid in enumerate(cl):
            lut[cid] = s
        seg = slice(ic * QF, (ic + 1) * QF)
        si[seg] = lut[tg[seg]]
    assert (si >= 0).all()

    rowsum_t = ch[ci, 0, col].astype(f)
    possum_t = ch[ci, 1 + si, col].astype(f)                    # incl self
    own_tb = ch[ci, 32 + si * 5 + bt, col].astype(f)            # incl self
    samet_b = ch[ci][np.arange(N)[:, None],
                     (32 + si * 5)[:, None] + np.arange(5)[None, :],
                     col[:, None]].sum(axis=1, dtype=f)

    diag_t = np.exp(scale_t * s_ii.astype(f))
    diag_b = np.exp(scale_b * s_ii.astype(f))

    cnt_t = np.bincount(tg, minlength=NT)
    n_tb = np.zeros((NT, NB), dtype=np.int64)
    np.add.at(n_tb, (tg, bt), 1)

    # ---- target SNN loss ----
    pos_t = possum_t - diag_t
    neg_t = rowsum_t - possum_t
    cnt_pos = cnt_t[tg]
    cnt_neg = N - cnt_pos
    valid_t = (cnt_pos >= 2) & (cnt_neg >= 1)
    pos_s = np.where(valid_t, pos_t, 1.0)
    neg_s = np.where(valid_t, neg_t, 1.0)
    loss_i = -np.log(pos_s / (pos_s + neg_s))
    lsum = np.bincount(tg, weights=np.where(valid_t, loss_i, 0.0), minlength=NT)
    vcnt = np.bincount(tg, weights=valid_t.astype(f), minlength=NT)
    mean = lsum / np.maximum(vcnt, 1.0)
    wt_w = np.asarray(weight_target).astype(f)
    loss_target = np.where(vcnt > 0, mean * wt_w, 0.0).sum()

    # ---- batch-effect loss ----
    pos_b = own_tb - diag_b
    neg_b = samet_b - own_tb
    cnt_pos_b = n_tb[tg, bt]
    cnt_neg_b = cnt_t[tg] - cnt_pos_b
    valid_b = (cnt_pos_b >= 2) & (cnt_neg_b >= 1)
    pos_bs = np.where(valid_b, pos_b, 1.0)
    neg_bs = np.where(valid_b, neg_b, 1.0)
    loss_bi = -np.log(pos_bs / (pos_bs + neg_bs))
    inv = np.where(valid_b, 1.0 / np.where(valid_b, loss_bi, 1.0), 0.0)
    lsum_b = np.bincount(bt, weights=inv, minlength=NB)
    vcnt_b = np.bincount(bt, weights=valid_b.astype(f), minlength=NB)
    mean_b = lsum_b / np.maximum(vcnt_b, 1.0)
    wb_w = np.asarray(weight_batch0).astype(f)
    loss_batch = np.where(vcnt_b > 0, mean_b * wb_w, 0.0).sum()

    return np.float32(0.9 * loss_target + 0.1 * loss_batch)


def _run_with_retry(nc, in_maps, core_ids, attempts=3):
    import time as _time

    from concourse.bass_utils import run_bass_kernel_spmd

    for i in range(attempts):
        try:
            return run_bass_kernel_spmd(nc, in_maps, core_ids)
        except Exception:
            if i == attempts - 1:
                raise
            _time.sleep(90)  # transient NRT device errors clear after a pause


def kernel(input, temperature, weight_target, weight_batch0, targets, batch0):
    global LAST_RESULT

    in_maps, meta = _host_prep(input, temperature, targets, batch0)
    nc = _get_compiled(meta["scale_t"], meta["scale_b"], meta["square_mode"],
                       meta["dspan"], meta["ncls"])

    LAST_RESULT = _run_with_retry(nc, in_maps, list(range(NCORES)))
    acc = np.stack([LAST_RESULT.results[c]["out"] for c in range(NCORES)])

    return _epilogue(acc, meta, weight_target, weight_batch0)
